# revision 11
# baseline (speedup 1.0000x reference)
"""LoRA generator kernel for Trainium2, sharded over 8 NeuronCores by layer.

Reference computation (see problem):
  pe = (condition @ W_proj + b_proj)                        (B=2, 224, 512)
  A  = (gelu(pe@WA1+bA1) @ WA2 + bA2) -> (B, L, 7, 16, 64)
  Bm = (gelu(pe@WB1+bB1) @ WB2 + bB2) -> (B, L, 7, 64, 16)
  out per (b, layer): concat over t of [tile_cols(A)*scA (16 x in_d),
                                        tile_rows(B)*scB (out_d x 16)]

Each core handles 4 layers (28 of the 224 projections). The big costs are
streaming its W_proj slice (22MB) in and writing its 36.8MB output slice; the
tiling/replication is done by DMA access patterns with step-0 (broadcast)
dims so the decoder outputs (56 rows x 4KB) fan out to ~37MB without compute.

pe is computed with the (tiny) condition as the stationary operand so the PE
array streams W_proj columns (84 big matmuls instead of 336 stationary
loads), then PE-transposed into pe_T (rows on the free axis) for the shared
decoder MLPs.
"""
import sys

sys.path.insert(0, "/opt/trn_rl_repo")

import numpy as np

import concourse.bass as bass
import concourse.bacc as bacc
import concourse.mybir as mybir
import concourse.tile as tile
from concourse.bass_utils import run_bass_kernel_spmd

F32 = mybir.dt.float32

NCORES = 8
NUM_LAYERS = 32
RANK = 16
PED = 512
EMB = 384
T = 7
L = NUM_LAYERS // NCORES          # 4 layers per core
LT = L * T                        # 28 projections per core
ROWS = 2 * LT                     # 56 rows (b, l, t); row = (l*7+t)*2 + b
WP_COLS = LT * PED                # 14336

IN_DS = [4096, 4096, 4096, 4096, 4096, 4096, 11008]
OUT_DS = [4096, 1024, 1024, 4096, 11008, 11008, 4096]
A_SIZES = [16 * d for d in IN_DS]
B_SIZES = [16 * d for d in OUT_DS]
LAYER_SIZE = sum(A_SIZES) + sum(B_SIZES)   # 1150976
OFF_A = []
OFF_B = []
_o = 0
for _t in range(T):
    OFF_A.append(_o)
    _o += A_SIZES[_t]
    OFF_B.append(_o)
    _o += B_SIZES[_t]
OUT_SZ = 2 * L * LAYER_SIZE

N_ROUNDS = 7                       # W_proj column rounds (4 lt-blocks each)
LT_PER_RD = LT // N_ROUNDS         # 4
RCOLS = LT_PER_RD * PED            # 2048


def _prow(row):
    """Physical partition of a row: 0..31 and 64..87 (spreads DMA ports)."""
    return row if row < 32 else 64 + (row - 32)


def _gbase(g):
    return g * 8 if g < 4 else 64 + (g - 4) * 8


def _build_nc():
    nc = bacc.Bacc(None, target_bir_lowering=False, debug=False)

    cond = nc.declare_dram_parameter("cond", [128, 6], F32, isOutput=False)
    wp = nc.declare_dram_parameter("wp", [EMB, WP_COLS], F32, isOutput=False)
    bpt = nc.declare_dram_parameter("bpt", [128, 4 * ROWS], F32, isOutput=False)
    wa1 = nc.declare_dram_parameter("wa1", [128, 1024], F32, isOutput=False)
    wb1 = nc.declare_dram_parameter("wb1", [128, 1024], F32, isOutput=False)
    wa2 = nc.declare_dram_parameter("wa2", [128, 2048], F32, isOutput=False)
    wb2 = nc.declare_dram_parameter("wb2", [128, 2048], F32, isOutput=False)
    ba1 = nc.declare_dram_parameter("ba1", [128, 2], F32, isOutput=False)
    bb1 = nc.declare_dram_parameter("bb1", [128, 2], F32, isOutput=False)
    sca = nc.declare_dram_parameter("sca", [128, ROWS], F32, isOutput=False)
    scb = nc.declare_dram_parameter("scb", [128, ROWS], F32, isOutput=False)
    sba2 = nc.declare_dram_parameter("sba2", [128, 1024], F32, isOutput=False)
    sbb2 = nc.declare_dram_parameter("sbb2", [128, 1024], F32, isOutput=False)
    ident = nc.declare_dram_parameter("ident", [128, 2], F32, isOutput=False)
    out = nc.declare_dram_parameter("out", [OUT_SZ], F32, isOutput=True)

    with tile.TileContext(nc) as tc:
        with (
            tc.tile_pool(name="const", bufs=1) as cpool,
            tc.tile_pool(name="wp", bufs=2) as wpool,
            tc.tile_pool(name="work", bufs=1) as wkpool,
            tc.tile_pool(name="pe2", bufs=2) as pe2pool,
            tc.tile_pool(name="ppe", bufs=1, space="PSUM") as ppe,
            tc.tile_pool(name="pmix", bufs=1, space="PSUM") as pmix,
        ):
            cond_sb = cpool.tile([128, 6], mybir.dt.float32r)
            nc.gpsimd.dma_start(cond_sb[:], cond[:])
            bpt_sb = cpool.tile([128, 4 * ROWS], F32)
            nc.sync.dma_start(bpt_sb[:], bpt[:])
            wa1_sb = cpool.tile([128, 1024], F32)
            nc.sync.dma_start(wa1_sb[:], wa1[:])
            wb1_sb = cpool.tile([128, 1024], F32)
            nc.sync.dma_start(wb1_sb[:], wb1[:])
            wa2_sb = cpool.tile([128, 2048], F32)
            nc.sync.dma_start(wa2_sb[:], wa2[:])
            wb2_sb = cpool.tile([128, 2048], F32)
            nc.sync.dma_start(wb2_sb[:], wb2[:])
            ba1_sb = cpool.tile([128, 2], F32)
            nc.sync.dma_start(ba1_sb[:], ba1[:])
            bb1_sb = cpool.tile([128, 2], F32)
            nc.sync.dma_start(bb1_sb[:], bb1[:])
            sca_sb = cpool.tile([128, ROWS], F32)
            nc.sync.dma_start(sca_sb[:], sca[:])
            scb_sb = cpool.tile([128, ROWS], F32)
            nc.sync.dma_start(scb_sb[:], scb[:])
            sba2_sb = cpool.tile([128, 1024], F32)
            nc.sync.dma_start(sba2_sb[:], sba2[:])
            sbb2_sb = cpool.tile([128, 1024], F32)
            nc.sync.dma_start(sbb2_sb[:], sbb2[:])
            ident_sb = cpool.tile([128, 2], F32)
            nc.sync.dma_start(ident_sb[:], ident[:])

            # ---- phase 1: pe = condition @ W_proj (cond stationary), then
            # PE-transpose into pe_T (PED on partitions, rows on free) ----
            psum_pe = [
                ppe.tile([128, ROWS], F32, tag=f"pe{mc}", name=f"psum_pe{mc}")
                for mc in range(4)
            ]
            for rd in range(N_ROUNDS):
                wp_t = []
                for kc in range(3):
                    t_ = wpool.tile([128, RCOLS], mybir.dt.float32r, tag=f"wp{kc}", name=f"wp_t{kc}")
                    nc.gpsimd.dma_start(
                        t_[:],
                        wp[kc * 128 : (kc + 1) * 128, rd * RCOLS : (rd + 1) * RCOLS],
                    )
                    wp_t.append(t_)
                pe2_sb = pe2pool.tile([2, RCOLS], F32, tag="pe2sb", name="pe2_sb")
                for ltl in range(LT_PER_RD):
                    p2 = pmix.tile([2, PED], F32, tag=f"mix{ltl % 4}", name="pe2_ps")
                    for kc in range(3):
                        nc.tensor.matmul(
                            p2[:],
                            cond_sb[:, kc * 2 : kc * 2 + 2],
                            wp_t[kc][:, ltl * PED : (ltl + 1) * PED],
                            start=(kc == 0),
                            stop=(kc == 2),
                        )
                    nc.scalar.copy(pe2_sb[:, ltl * PED : (ltl + 1) * PED], p2[:])
                for ltl in range(LT_PER_RD):
                    lt = rd * LT_PER_RD + ltl
                    for mc in range(4):
                        nc.tensor.transpose(
                            psum_pe[mc][:, 2 * lt : 2 * lt + 2],
                            pe2_sb[:, ltl * PED + mc * 128 : ltl * PED + (mc + 1) * 128],
                            ident_sb[0:2, 0:2],
                        )
            pe_sb = []
            for mc in range(4):
                t_ = wkpool.tile([128, ROWS], F32, tag=f"pe_sb{mc}", name=f"pe_sb{mc}")
                nc.vector.tensor_add(
                    t_[:], psum_pe[mc][:], bpt_sb[:, mc * ROWS : (mc + 1) * ROWS]
                )
                pe_sb.append(t_)

            # ---- phase 2: decoders ----
            PGROUPS = [(0, 0, 32), (64, 32, 24)]  # (psum partition base, col0, M)

            def decoder(w1_sb, b1_sb, w2_sb, sc_sb, name):
                h_sb = []
                for mc in range(2):
                    hp = pmix.tile([128, ROWS], F32, tag=f"mix{mc}", name=f"hp{mc}")
                    for kc in range(4):
                        nc.tensor.matmul(
                            hp[:],
                            w1_sb[:, kc * 256 + mc * 128 : kc * 256 + (mc + 1) * 128],
                            pe_sb[kc][:],
                            start=(kc == 0),
                            stop=(kc == 3),
                        )
                    hs = wkpool.tile(
                        [128, ROWS], F32, tag=f"h_sb{name}{mc}", name=f"hs_{name}{mc}"
                    )
                    nc.scalar.activation(
                        hs[:], hp[:], mybir.ActivationFunctionType.Gelu,
                        bias=b1_sb[:, mc : mc + 1],
                    )
                    nc.vector.tensor_mul(hs[:], hs[:], sc_sb[:])
                    h_sb.append(hs)
                opsum = []
                for nh in range(2):
                    op = pmix.tile([128, 512], F32, tag=f"mix{2 + nh}", name=f"opsum{nh}")
                    for pbase, c0, M in PGROUPS:
                        for kc in range(2):
                            nc.tensor.matmul(
                                op[pbase : pbase + M, :],
                                h_sb[kc][:, c0 : c0 + M],
                                w2_sb[:, kc * 1024 + nh * 512 : kc * 1024 + (nh + 1) * 512],
                                start=(kc == 0),
                                stop=(kc == 1),
                            )
                    opsum.append(op)
                return opsum

            # decoder A -> oa (128, 1024): oa[prow, r*64+c] = scA*A[row, r, c] + bias
            opsum_a = decoder(wa1_sb, ba1_sb, wa2_sb, sca_sb, "a")
            oa = wkpool.tile([128, 4096], F32)
            for nh in range(2):
                for pbase, c0, M in PGROUPS:
                    src = opsum_a[nh][pbase : pbase + M, :].rearrange(
                        "p (r c) -> p r c", c=64
                    )
                    bias = sba2_sb[
                        pbase : pbase + M, nh * 512 : (nh + 1) * 512
                    ].rearrange("p (r c) -> p r c", c=64)
                    for d in range(4):
                        dst = oa[
                            pbase : pbase + M, nh * 2048 : (nh + 1) * 2048
                        ].rearrange("p (r x) -> p r x", x=256)[:, :, d * 64 : (d + 1) * 64]
                        nc.vector.tensor_add(dst, src, bias)

            # decoder B -> ob_sb (128, 1024): ob_sb[prow, j] = scB * Bvec[row, j]
            opsum_b = decoder(wb1_sb, bb1_sb, wb2_sb, scb_sb, "b")
            ob_sb = wkpool.tile([128, 1024], F32)
            for nh in range(2):
                for pbase, c0, M in PGROUPS:
                    nc.vector.tensor_add(
                        ob_sb[pbase : pbase + M, nh * 512 : (nh + 1) * 512],
                        opsum_b[nh][pbase : pbase + M, :],
                        sbb2_sb[pbase : pbase + M, nh * 512 : (nh + 1) * 512],
                    )

            # ---- phase 3: rearrange into engine-striding exp layouts ----
            # aexp[r*8 + rowloc, g*1024 + du] = oa[prow(g*8+rowloc), r*64 + du%64]
            #   (so one A piece's descriptors stride partitions 8 apart -> all
            #    16 SDMA engines). Built: dup-x4 initial copy, then column
            #    doubling 256->512->1024.
            # bexp[k*8 + slot, g*1024 + j] = ob[prow(g*8+slot), j] for all k
            #   (full 1024-vec replicated on 16 partitions). Built: one copy
            #    per g, then partition-range doubling 8->16->32->64->128.
            aexp = wkpool.tile([128, T * 1024], F32)
            bexp = wkpool.tile([128, T * 1024], F32)
            pa = oa[:, :].ap[0][0]
            pax = aexp[:, :].ap[0][0]
            pob = ob_sb[:, :].ap[0][0]
            pbx = bexp[:, :].ap[0][0]
            oa_t = oa[:, :].tensor
            aexp_t = aexp[:, :].tensor
            ob_t = ob_sb[:, :].tensor
            bexp_t = bexp[:, :].tensor
            rearr_engs = [nc.scalar, nc.gpsimd]
            ei = 0
            for g in range(T):
                gb = _gbase(g)
                for r in range(16):
                    dst = bass.AP(aexp_t, r * 8 * pax + g * 1024, [[pax, 8], [1, 256]])
                    src = bass.AP(oa_t, gb * pa + r * 256, [[pa, 8], [1, 256]])
                    rearr_engs[ei % 2].dma_start(dst, src)
                    ei += 1
                dstb = bass.AP(bexp_t, g * 1024, [[pbx, 8], [1, 1024]])
                srcb = bass.AP(ob_t, gb * pob, [[pob, 8], [1, 1024]])
                rearr_engs[ei % 2].dma_start(dstb, srcb)
                ei += 1
            # aexp column doubling (dup4 unit 256 -> 512 -> 1024), per g
            for g in range(T):
                for w in (256, 512):
                    dst = bass.AP(aexp_t, g * 1024 + w, [[pax, 128], [1, w]])
                    src = bass.AP(aexp_t, g * 1024, [[pax, 128], [1, w]])
                    rearr_engs[ei % 2].dma_start(dst, src)
                    ei += 1
            # bexp partition doubling (all columns at once)
            for np_ in (8, 16, 32, 64):
                dst = bass.AP(bexp_t, np_ * pbx, [[pbx, np_], [1, T * 1024]])
                src = bass.AP(bexp_t, 0, [[pbx, np_], [1, T * 1024]])
                nc.gpsimd.dma_start(dst, src)

            # ---- phase 4: piece DMAs (the ~37MB fan-out, 4KB descriptors) ----
            for row in range(ROWS):
                lt, b = row // 2, row % 2
                l, t = lt // T, lt % T
                g, rowloc = row // 8, row % 8
                in_d, out_d = IN_DS[t], OUT_DS[t]
                # A piece (16, in_d): nf full 1KB reps + optional 768-elem tail
                base = (b * L + l) * LAYER_SIZE + OFF_A[t]
                nf, tail = in_d // 1024, in_d % 1024
                dst = bass.AP(out, base, [[in_d, 16], [1024, nf], [1, 1024]])
                src = bass.AP(
                    aexp_t, rowloc * pax + g * 1024, [[8 * pax, 16], [0, nf], [1, 1024]]
                )
                nc.sync.dma_start(dst, src)
                if tail:
                    dst = bass.AP(out, base + nf * 1024, [[in_d, 16], [1, tail]])
                    src = bass.AP(
                        aexp_t, rowloc * pax + g * 1024, [[8 * pax, 16], [1, tail]]
                    )
                    nc.sync.dma_start(dst, src)
                # B piece (out_d, 16) = nb reps of the 1024-vec
                base = (b * L + l) * LAYER_SIZE + OFF_B[t]
                nb = out_d // 64
                nbf, nbt = nb // 16, nb % 16
                dst = bass.AP(
                    out, base, [[1024, 16], [16 * 1024, nbf], [1, 1024]]
                )
                src = bass.AP(
                    bexp_t, rowloc * pbx + g * 1024, [[8 * pbx, 16], [0, nbf], [1, 1024]]
                )
                nc.sync.dma_start(dst, src)
                if nbt:
                    dst = bass.AP(
                        out, base + nbf * 16 * 1024, [[1024, nbt], [1, 1024]]
                    )
                    src = bass.AP(
                        bexp_t, rowloc * pbx + g * 1024, [[8 * pbx, nbt], [1, 1024]]
                    )
                    nc.sync.dma_start(dst, src)

    nc.finalize()
    return nc


_NC = None


def _get_nc():
    global _NC
    if _NC is None:
        _NC = _build_nc()
    return _NC


def _marshal(inputs):
    """Build the per-core input maps from full inputs."""
    condition = np.asarray(inputs["condition"], np.float32)
    W_proj = np.asarray(inputs["W_proj"], np.float32)
    b_proj = np.asarray(inputs["b_proj"], np.float32)
    WA1 = np.asarray(inputs["WA1"], np.float32)
    bA1 = np.asarray(inputs["bA1"], np.float32)
    WA2 = np.asarray(inputs["WA2"], np.float32)
    bA2 = np.asarray(inputs["bA2"], np.float32)
    WB1 = np.asarray(inputs["WB1"], np.float32)
    bB1 = np.asarray(inputs["bB1"], np.float32)
    WB2 = np.asarray(inputs["WB2"], np.float32)
    bB2 = np.asarray(inputs["bB2"], np.float32)
    scales = np.asarray(inputs["scales"], np.float32)

    # shared (replicated) arrangements
    cond_arr = np.zeros((128, 6), np.float32)
    for kc in range(3):
        cond_arr[:, kc * 2 : kc * 2 + 2] = condition[:, kc * 128 : (kc + 1) * 128].T
    wa1_arr = np.zeros((128, 1024), np.float32)
    wb1_arr = np.zeros((128, 1024), np.float32)
    for kc in range(4):
        wa1_arr[:, kc * 256 : (kc + 1) * 256] = WA1[kc * 128 : (kc + 1) * 128, :]
        wb1_arr[:, kc * 256 : (kc + 1) * 256] = WB1[kc * 128 : (kc + 1) * 128, :]
    wa2_arr = np.zeros((128, 2048), np.float32)
    wb2_arr = np.zeros((128, 2048), np.float32)
    for kc in range(2):
        wa2_arr[:, kc * 1024 : (kc + 1) * 1024] = WA2[kc * 128 : (kc + 1) * 128, :]
        wb2_arr[:, kc * 1024 : (kc + 1) * 1024] = WB2[kc * 128 : (kc + 1) * 128, :]
    ba1_arr = np.ascontiguousarray(bA1.reshape(2, 128).T)
    bb1_arr = np.ascontiguousarray(bB1.reshape(2, 128).T)
    ident_arr = np.zeros((128, 2), np.float32)
    ident_arr[0, 0] = 1.0
    ident_arr[1, 1] = 1.0

    in_maps = []
    for c in range(NCORES):
        lt0 = c * LT
        wp_c = np.ascontiguousarray(W_proj[:, lt0 * PED : (lt0 + LT) * PED])
        bp_c = b_proj[lt0 * PED : (lt0 + LT) * PED].reshape(LT, 4, 128)
        bpt_arr = np.zeros((128, 4 * ROWS), np.float32)
        sca_row = np.zeros(ROWS, np.float32)
        scb_row = np.zeros(ROWS, np.float32)
        for row in range(ROWS):
            lt, b = row // 2, row % 2
            for mc in range(4):
                bpt_arr[:, mc * ROWS + row] = bp_c[lt, mc, :]
            sca_row[row] = scales[lt0 + lt, 0]
            scb_row[row] = scales[lt0 + lt, 1]
        sca_arr = np.broadcast_to(sca_row[None, :], (128, ROWS)).copy()
        scb_arr = np.broadcast_to(scb_row[None, :], (128, ROWS)).copy()
        sba2_arr = np.zeros((128, 1024), np.float32)
        sbb2_arr = np.zeros((128, 1024), np.float32)
        for row in range(ROWS):
            p = _prow(row)
            sba2_arr[p, :] = sca_row[row] * bA2
            sbb2_arr[p, :] = scb_row[row] * bB2
        in_maps.append(
            {
                "cond": cond_arr,
                "wp": wp_c,
                "bpt": bpt_arr,
                "wa1": wa1_arr,
                "wb1": wb1_arr,
                "wa2": wa2_arr,
                "wb2": wb2_arr,
                "ba1": ba1_arr,
                "bb1": bb1_arr,
                "sca": sca_arr,
                "scb": scb_arr,
                "sba2": sba2_arr,
                "sbb2": sbb2_arr,
                "ident": ident_arr,
            }
        )
    return in_maps


def _ensure_ntff_hook():
    """Register the axon NTFF profile hook if the boot didn't (module was
    missing at boot time)."""
    import types

    ah = sys.modules.get("antenv.axon_hooks")
    if ah is None:
        ah = types.ModuleType("antenv.axon_hooks")
        ah._hook = None

        def _set(h, _m=ah):
            _m._hook = h

        def _get(_m=ah):
            return _m._hook

        ah.set_axon_ntff_profile_hook = _set
        ah.get_axon_ntff_profile_hook = _get
        sys.modules["antenv.axon_hooks"] = ah
        import antenv

        antenv.axon_hooks = ah
    if ah.get_axon_ntff_profile_hook() is None:
        if "/root/.axon_site" not in sys.path:
            sys.path.insert(0, "/root/.axon_site")
        from trn_agent_boot.trn_boot import _ntff_profile_via_ctypes

        hook = _ntff_profile_via_ctypes("/opt/axon/libaxon_pjrt.so")
        if hook is not None:
            ah.set_axon_ntff_profile_hook(hook)


def _run(inputs, trace=False):
    if trace:
        _ensure_ntff_hook()
    nc = _get_nc()
    in_maps = _marshal(inputs)
    res = run_bass_kernel_spmd(nc, in_maps, list(range(NCORES)), trace=trace)
    full = np.empty((2, NUM_LAYERS, LAYER_SIZE), np.float32)
    for c in range(NCORES):
        full[:, c * L : (c + 1) * L, :] = res.results[c]["out"].reshape(
            2, L, LAYER_SIZE
        )
    return full.reshape(2, -1), res


def kernel(**inputs) -> np.ndarray:
    out, _ = _run(inputs, trace=False)
    return out
